# revision 11
# baseline (speedup 1.0000x reference)
"""GIN message-passing + MLP head, 8-core SPMD Trainium2 (Bass/Tile) kernel.

Strategy:
- Nodes sharded by dst across 8 cores (7680 padded nodes/core). Full x
  (bf16) replicated per core for edge gathers -> no collectives needed.
- agg = segment_sum(x[src], dst) computed ONCE (shared by both GIN convs):
  edges sorted by 512-node dst window; x[src] rows fetched with gpsimd
  dma_gather (int16 idx => split lo/hi src halves); scatter-add done as
  PE matmuls against a one-hot dst-offset matrix S built on DVE.
- MLP head: BN folded into first conv weights on host; bf16 matmuls with
  fp32 PSUM accumulation; ACT-engine epilogues (bias + Relu/Lrelu/Sigmoid);
  activations stored feature-major [feat, nodes]; big intermediates staged
  through internal DRAM; l1/l2 weights SBUF-resident per output half.
"""
import os
import sys
import types
from contextlib import ExitStack

import numpy as np
import ml_dtypes

import concourse.bass as bass
import concourse.tile as tile
from concourse import bacc, mybir
from concourse.bass_utils import run_bass_kernel_spmd

BF16 = ml_dtypes.bfloat16

# ---- problem constants (hardcoded; must match the reference) ----
N_NODES = 60000
N_EDGES = 960000
C = 128               # IN_C
HID = 512
DCAT = 1024
DFC = 2048
DL1 = 4096
OUTC = 64
BN_EPS = 1e-5
LEAKY = 0.01

N_CORES = 8
NODES = 7680          # padded nodes per core
NPAD = N_CORES * NODES  # 61440
WIN = 512             # dst window (node tile) size
NWIN = NODES // WIN   # 15 windows (= node tiles) per core
NWIN_G = NPAD // WIN  # 120 global windows
LO_ROWS = 32768       # src < LO_ROWS gathers from x[0:LO_ROWS]

FP32 = mybir.dt.float32
BF = mybir.dt.bfloat16
I16 = mybir.dt.int16
AF = mybir.ActivationFunctionType

LAST_RESULT = None          # test harness introspection
_PROGRAM_CACHE = {}
K_PHASES = int(os.environ.get("K_PHASES", "6"))  # debug: stop after phase N


def _install_ntff_shim():
    """antenv.axon_hooks is missing in this image; shim it so trace=True
    (BASS_TRACE=1) can capture NTFF profiles. No-op if already present."""
    try:
        import antenv.axon_hooks  # noqa: F401
        return
    except ImportError:
        pass
    try:
        import antenv
        from trn_agent_boot.trn_boot import _ntff_profile_via_ctypes
        mod = types.ModuleType("antenv.axon_hooks")
        mod._hook = _ntff_profile_via_ctypes("/opt/axon/libaxon_pjrt.so")
        mod.get_axon_ntff_profile_hook = lambda: mod._hook
        mod.set_axon_ntff_profile_hook = lambda h: setattr(mod, "_hook", h)
        sys.modules["antenv.axon_hooks"] = mod
        antenv.axon_hooks = mod
    except Exception:
        pass


# ---------------------------------------------------------------------------
# device program
# ---------------------------------------------------------------------------

def build_program(t_lo: int, t_hi: int):
    nc = bacc.Bacc("TRN2", target_bir_lowering=False, debug=False,
                   num_devices=N_CORES, num_swdge_queues=4)
    T = t_lo + t_hi

    # ---- I/O ----
    xg = nc.dram_tensor("xg", [NPAD, C], BF, kind="ExternalInput").ap()
    xt = nc.dram_tensor("xt", [C, NODES], BF, kind="ExternalInput").ap()
    gilo = nc.dram_tensor("gilo", [NWIN, 128, t_lo * 8], I16,
                          kind="ExternalInput").ap()
    gihi = nc.dram_tensor("gihi", [NWIN, 128, t_hi * 8], I16,
                          kind="ExternalInput").ap()
    doff = nc.dram_tensor("doff", [NWIN, 128, T], FP32,
                          kind="ExternalInput").ap()
    w1a = nc.dram_tensor("w1a", [C, HID], BF, kind="ExternalInput").ap()
    w2a = nc.dram_tensor("w2a", [HID, HID], BF, kind="ExternalInput").ap()
    w1b = nc.dram_tensor("w1b", [C, HID], BF, kind="ExternalInput").ap()
    w2b = nc.dram_tensor("w2b", [HID, HID], BF, kind="ExternalInput").ap()
    fcw = nc.dram_tensor("fcw", [DCAT, DFC], BF, kind="ExternalInput").ap()
    l1w = nc.dram_tensor("l1w", [DFC, DL1], BF, kind="ExternalInput").ap()
    l2w = nc.dram_tensor("l2w", [DL1, DFC], BF, kind="ExternalInput").ap()
    outw = nc.dram_tensor("outw", [DFC, 128], BF, kind="ExternalInput").ap()
    # biases, per-partition layout [128, out_blocks]
    b1a = nc.dram_tensor("b1a", [128, 4], FP32, kind="ExternalInput").ap()
    b2a = nc.dram_tensor("b2a", [128, 4], FP32, kind="ExternalInput").ap()
    b1b = nc.dram_tensor("b1b", [128, 4], FP32, kind="ExternalInput").ap()
    b2b = nc.dram_tensor("b2b", [128, 4], FP32, kind="ExternalInput").ap()
    fcb = nc.dram_tensor("fcb", [128, 16], FP32, kind="ExternalInput").ap()
    l1b = nc.dram_tensor("l1b", [128, 32], FP32, kind="ExternalInput").ap()
    l2b = nc.dram_tensor("l2b", [128, 16], FP32, kind="ExternalInput").ap()
    outb = nc.dram_tensor("outb", [128, 1], FP32, kind="ExternalInput").ap()

    y = nc.dram_tensor("y", [128, NODES], FP32, kind="ExternalOutput").ap()

    # internal DRAM staging, feature-block-major [kb, 128, nodes]
    cat = nc.dram_tensor("cat", [8, 128, NODES], BF).ap()
    g1 = nc.dram_tensor("g1", [16, 128, NODES], BF).ap()
    g2 = nc.dram_tensor("g2", [32, 128, NODES], BF).ap()
    g3 = nc.dram_tensor("g3", [16, 128, NODES], BF).ap()

    def wview(ap, kb):
        # [K, O] dram -> [128, kb, O] AP (partition = K within block)
        return ap.rearrange("(kb p) o -> p kb o", p=128, kb=kb)

    with tile.TileContext(nc) as tc, ExitStack() as big:
        ps_pool = big.enter_context(tc.tile_pool(name="ps", bufs=8,
                                                 space="PSUM"))
        bias_pool = big.enter_context(tc.tile_pool(name="bias", bufs=1))
        otile_pool = big.enter_context(tc.tile_pool(name="otile", bufs=6))

        def load_bias(ap, nb):
            t = bias_pool.tile([128, nb], FP32, tag=ap.name)
            nc.sync.dma_start(t[:], ap[:])
            return t

        b1a_t = load_bias(b1a, 4)
        b2a_t = load_bias(b2a, 4)
        b1b_t = load_bias(b1b, 4)
        b2b_t = load_bias(b2b, 4)
        fcb_t = load_bias(fcb, 16)
        l1b_t = load_bias(l1b, 32)
        l2b_t = load_bias(l2b, 16)
        outb_t = load_bias(outb, 1)

        with ExitStack() as ph12:
            persist = ph12.enter_context(tc.tile_pool(name="persist",
                                                      bufs=1))
            xt_t = persist.tile([128, NODES], BF, tag="xt")
            nc.sync.dma_start(xt_t[:], xt[:])
            h_t = persist.tile([128, NODES], BF, tag="h")

            # ======== phase 1: aggregation  h = x + scatter_add ========
            for _ph1 in ([0] if K_PHASES >= 1 else []):
             with ExitStack() as ph:
                ipool = ph.enter_context(tc.tile_pool(name="aggidx",
                                                      bufs=3))
                gpool = ph.enter_context(tc.tile_pool(name="gath", bufs=3))
                spool = ph.enter_context(tc.tile_pool(name="onehot",
                                                      bufs=6))
                cpool = ph.enter_context(tc.tile_pool(name="aggconst",
                                                      bufs=1))

                qn = [0]
                iota_t = cpool.tile([128, WIN], I16)
                nc.gpsimd.iota(iota_t[:], pattern=[[1, WIN]], base=0,
                               channel_multiplier=0)

                for w in range(NWIN):
                    il = ipool.tile([128, t_lo * 8], I16, tag="il")
                    nc.sync.dma_start(il[:], gilo[w])
                    ih = ipool.tile([128, t_hi * 8], I16, tag="ih")
                    nc.sync.dma_start(ih[:], gihi[w])
                    do_t = ipool.tile([128, T], FP32, tag="doff")
                    nc.sync.dma_start(do_t[:], doff[w])

                    # SWDGE ring holds 1024 descriptors -> chunk gathers
                    # to <=8 edge-tiles (1024 idxs) per instruction.
                    gl = gpool.tile([128, t_lo, C], BF, tag="gl")
                    for c0 in range(0, t_lo, 8):
                        c1 = min(c0 + 8, t_lo)
                        ni = (c1 - c0) * 128
                        nc.gpsimd.dma_gather(
                            gl[:, c0:c1, :], xg[0:LO_ROWS, :],
                            il[:, c0 * 8:c1 * 8],
                            num_idxs=ni, num_idxs_reg=ni, elem_size=C,
                            queue_num=qn[0] % 4)
                        qn[0] += 1
                    gh = gpool.tile([128, t_hi, C], BF, tag="gh")
                    for c0 in range(0, t_hi, 8):
                        c1 = min(c0 + 8, t_hi)
                        ni = (c1 - c0) * 128
                        nc.gpsimd.dma_gather(
                            gh[:, c0:c1, :], xg[LO_ROWS:NPAD, :],
                            ih[:, c0 * 8:c1 * 8],
                            num_idxs=ni, num_idxs_reg=ni, elem_size=C,
                            queue_num=qn[0] % 4)
                        qn[0] += 1

                    ps = ps_pool.tile([128, WIN], FP32, tag="ps")
                    for t in range(T):
                        s = spool.tile([128, WIN], BF, tag="s")
                        nc.vector.tensor_scalar(
                            s[:], iota_t[:], do_t[:, t:t + 1], None,
                            mybir.AluOpType.is_equal)
                        g = gl[:, t, :] if t < t_lo else gh[:, t - t_lo, :]
                        nc.tensor.matmul(ps[:], g, s[:],
                                         start=(t == 0), stop=(t == T - 1))
                    nc.vector.tensor_add(h_t[:, w * WIN:(w + 1) * WIN],
                                         xt_t[:, w * WIN:(w + 1) * WIN],
                                         ps[:])

            # ======== phase 2: the two GIN conv MLPs -> cat ========
            for _ph2 in ([0] if K_PHASES >= 2 else []):
             with ExitStack() as ph:
                wpool = ph.enter_context(tc.tile_pool(name="convw", bufs=1))
                apool = ph.enter_context(tc.tile_pool(name="convact",
                                                      bufs=2))

                w1a_t = wpool.tile([128, HID], BF, tag="w1a")
                nc.sync.dma_start(w1a_t[:], w1a[:])
                w2a_t = wpool.tile([128, 4, HID], BF, tag="w2a")
                nc.sync.dma_start(w2a_t[:], wview(w2a, 4))
                w1b_t = wpool.tile([128, HID], BF, tag="w1b")
                nc.sync.dma_start(w1b_t[:], w1b[:])
                w2b_t = wpool.tile([128, 4, HID], BF, tag="w2b")
                nc.sync.dma_start(w2b_t[:], wview(w2b, 4))

                for nt in range(NWIN):
                    hs = h_t[:, nt * WIN:(nt + 1) * WIN]
                    for conv, (w1_t, w2_t, bb1, bb2) in enumerate(
                            [(w1a_t, w2a_t, b1a_t, b2a_t),
                             (w1b_t, w2b_t, b1b_t, b2b_t)]):
                        xa = apool.tile([128, 4, WIN], BF, tag="x1a")
                        for of in range(4):
                            ps = ps_pool.tile([128, WIN], FP32, tag="ps")
                            nc.tensor.matmul(
                                ps[:],
                                w1_t[:, of * 128:(of + 1) * 128],
                                hs, start=True, stop=True)
                            nc.scalar.activation(xa[:, of, :], ps[:],
                                                 AF.Relu,
                                                 bias=bb1[:, of:of + 1])
                        for of in range(4):
                            ps = ps_pool.tile([128, WIN], FP32, tag="ps")
                            for kb in range(4):
                                nc.tensor.matmul(
                                    ps[:],
                                    w2_t[:, kb, of * 128:(of + 1) * 128],
                                    xa[:, kb, :],
                                    start=(kb == 0), stop=(kb == 3))
                            ot = otile_pool.tile([128, WIN], BF, tag="ot")
                            nc.scalar.activation(ot[:], ps[:], AF.Relu,
                                                 bias=bb2[:, of:of + 1])
                            nc.sync.dma_start(
                                cat[conv * 4 + of, :,
                                    nt * WIN:(nt + 1) * WIN], ot[:])

        # big weight pool created only now so agg/conv phases keep SBUF room
        bigw = big.enter_context(tc.tile_pool(name="bigw", bufs=2))
        linpool = None

        # ============ generic streamed linear layer ============
        def _epilogue(ps, dst, of_g, bias_t, func, alpha, nts, out_fp32_to):
            if out_fp32_to is not None:
                ot = otile_pool.tile([128, WIN], FP32, tag="yt")
                nc.scalar.activation(ot[:], ps[:], func,
                                     bias=bias_t[:, of_g:of_g + 1],
                                     alpha=alpha)
                nc.sync.dma_start(out_fp32_to[:, nts], ot[:])
            else:
                ot = otile_pool.tile([128, WIN], BF, tag="ot")
                nc.scalar.activation(ot[:], ps[:], func,
                                     bias=bias_t[:, of_g:of_g + 1],
                                     alpha=alpha)
                nc.sync.dma_start(dst[of_g, :, nts], ot[:])

        def linear(src, dst, w_t, bias_t, of_base, kb_n, of_n, func,
                   in_tag, kgroups=1, alpha=0.0, out_fp32_to=None):
            kb_per_g = kb_n // kgroups
            for nt in range(NWIN):
                nts = slice(nt * WIN, (nt + 1) * WIN)
                if kgroups == 1:
                    it = linpool.tile([128, kb_n, WIN], BF, tag=in_tag)
                    nc.sync.dma_start(it[:],
                                      src[:, :, nts].transpose([1, 0, 2]))
                    for of in range(of_n):
                        ps = ps_pool.tile([128, WIN], FP32, tag="ps")
                        for kb in range(kb_n):
                            nc.tensor.matmul(
                                ps[:],
                                w_t[:, kb, of * 128:(of + 1) * 128],
                                it[:, kb, :],
                                start=(kb == 0), stop=(kb == kb_n - 1))
                        _epilogue(ps, dst, of_base + of, bias_t, func,
                                  alpha, nts, out_fp32_to)
                else:
                    pss = [ps_pool.tile([128, WIN], FP32, tag="ps",
                                        name=f"pss{of}")
                           for of in range(of_n)]
                    for kg in range(kgroups):
                        it = linpool.tile([128, kb_per_g, WIN], BF,
                                          tag=in_tag)
                        ksl = slice(kg * kb_per_g, (kg + 1) * kb_per_g)
                        nc.sync.dma_start(
                            it[:], src[ksl, :, nts].transpose([1, 0, 2]))
                        for of in range(of_n):
                            for kb in range(kb_per_g):
                                kbg = kg * kb_per_g + kb
                                nc.tensor.matmul(
                                    pss[of][:],
                                    w_t[:, kbg, of * 128:(of + 1) * 128],
                                    it[:, kb, :],
                                    start=(kbg == 0),
                                    stop=(kbg == kb_n - 1))
                    for of in range(of_n):
                        _epilogue(pss[of], dst, of_base + of, bias_t, func,
                                  alpha, nts, out_fp32_to)

        # ---- fc: cat[1024] -> g1[2048], leaky relu ----
        for _ph3 in ([0] if K_PHASES >= 3 else []):
         with ExitStack() as ph:
            linpool = ph.enter_context(tc.tile_pool(name="fcin", bufs=2))
            fcw_t = bigw.tile([128, 8, DFC], BF, tag="bigw")
            nc.sync.dma_start(fcw_t[:], wview(fcw, 8))
            linear(cat, g1, fcw_t, fcb_t, 0, 8, 16, AF.Lrelu, "fcin",
                   alpha=LEAKY)

        # ---- l1: g1[2048] -> g2[4096], linear (two output halves) ----
        for _ph4 in ([0] if K_PHASES >= 4 else []):
         with ExitStack() as ph:
            linpool = ph.enter_context(tc.tile_pool(name="l1in", bufs=2))
            for hx in range(2):
                wt = bigw.tile([128, 16, 2048], BF, tag="bigw")
                nc.sync.dma_start(
                    wt[:], wview(l1w, 16)[:, :, hx * 2048:(hx + 1) * 2048])
                linear(g1, g2, wt, l1b_t, hx * 16, 16, 16, AF.Identity,
                       "l1in")

        # ---- l2: g2[4096] -> g3[2048], linear (two output halves) ----
        for _ph5 in ([0] if K_PHASES >= 5 else []):
         with ExitStack() as ph:
            linpool = ph.enter_context(tc.tile_pool(name="l2in", bufs=3))
            for hx in range(2):
                wt = bigw.tile([128, 32, 1024], BF, tag="bigw")
                nc.sync.dma_start(
                    wt[:], wview(l2w, 32)[:, :, hx * 1024:(hx + 1) * 1024])
                linear(g2, g3, wt, l2b_t, hx * 8, 32, 8, AF.Identity,
                       "l2in", kgroups=2)

        # ---- out: g3[2048] -> y[64(pad128)], sigmoid, fp32 ----
        for _ph6 in ([0] if K_PHASES >= 6 else []):
         with ExitStack() as ph:
            linpool = ph.enter_context(tc.tile_pool(name="outin", bufs=2))
            ow_t = bigw.tile([128, 16, 128], BF, tag="outw")
            nc.sync.dma_start(ow_t[:], wview(outw, 16))
            linear(g3, None, ow_t, outb_t, 0, 16, 1, AF.Sigmoid, "outin",
                   out_fp32_to=y)

    nc.compile()
    return nc


# ---------------------------------------------------------------------------
# host-side sharding / preprocessing
# ---------------------------------------------------------------------------

def _fold_bn(w1, b1, gamma, beta, mean, var):
    s = (np.asarray(gamma, np.float64)
         / np.sqrt(np.asarray(var, np.float64) + BN_EPS))
    w = np.asarray(w1, np.float64) * s[None, :]
    b = (np.asarray(b1, np.float64) - np.asarray(mean, np.float64)) * s \
        + np.asarray(beta, np.float64)
    return w.astype(BF16), b.astype(np.float32)


def _bias_tile(b, nb):
    # [out] -> [128, nb] with b[of*128+p] at [p, of]
    bb = np.asarray(b, np.float32)
    return np.ascontiguousarray(bb.reshape(nb, 128).T)


def _idx_layout(arr, nwin):
    # [nwin, T*128] int16 -> [nwin, 128, T*8]: idx i at partition i%16,
    # col i//16, replicated over the 8 gpsimd-core partition groups.
    t8 = arr.shape[1] // 16
    a = arr.reshape(nwin, t8, 16).transpose(0, 2, 1)       # [nwin, 16, T*8]
    return np.ascontiguousarray(np.tile(a, (1, 8, 1)))     # [nwin, 128, T*8]


def kernel(**inputs) -> np.ndarray:
    global LAST_RESULT
    x = np.asarray(inputs["x"], np.float32)
    edge_index = np.asarray(inputs["edge_index"])

    # ---- graph partition: sort edges by (global dst window, src half) ----
    src = edge_index[0].astype(np.int64)
    dst = edge_index[1].astype(np.int64)
    win = dst // WIN                      # 0..119 global window
    half = (src >= LO_ROWS).astype(np.int64)
    key = win * 2 + half
    order = np.argsort(key, kind="stable")
    src_s = src[order]
    dst_s = dst[order]
    key_s = key[order]
    counts = np.bincount(key_s, minlength=NWIN_G * 2)
    starts = np.zeros(NWIN_G * 2 + 1, np.int64)
    np.cumsum(counts, out=starts[1:])
    rank = np.arange(N_EDGES, dtype=np.int64) - starts[key_s]

    cnt_lo = counts[0::2]
    cnt_hi = counts[1::2]
    t_lo = int(np.ceil(cnt_lo.max() / 128))
    t_hi = int(np.ceil(cnt_hi.max() / 128))
    T = t_lo + t_hi

    gidx_lo = np.zeros((NWIN_G, t_lo * 128), np.int16)
    gidx_hi = np.zeros((NWIN_G, t_hi * 128), np.int16)
    dstoff = np.full((NWIN_G, T * 128), -1.0, np.float32)

    is_lo = key_s % 2 == 0
    wl = key_s[is_lo] // 2
    gidx_lo[wl, rank[is_lo]] = src_s[is_lo].astype(np.int16)
    dstoff[wl, rank[is_lo]] = (dst_s[is_lo] % WIN).astype(np.float32)
    wh = key_s[~is_lo] // 2
    gidx_hi[wh, rank[~is_lo]] = (src_s[~is_lo] - LO_ROWS).astype(np.int16)
    dstoff[wh, t_lo * 128 + rank[~is_lo]] = \
        (dst_s[~is_lo] % WIN).astype(np.float32)

    gilo = _idx_layout(gidx_lo, NWIN_G)
    gihi = _idx_layout(gidx_hi, NWIN_G)
    # dstoff: edge t*128+j -> [w, j, t]
    doff = np.ascontiguousarray(
        dstoff.reshape(NWIN_G, T, 128).transpose(0, 2, 1))

    # ---- node features ----
    xpad = np.zeros((NPAD, C), np.float32)
    xpad[:N_NODES] = x
    xg = xpad.astype(BF16)
    xt_all = np.ascontiguousarray(xg.T)   # [128, NPAD] bf16

    # ---- weights (fold BN, cast bf16) ----
    w1a, b1a = _fold_bn(inputs["w1a"], inputs["b1a"], inputs["gamma_a"],
                        inputs["beta_a"], inputs["mean_a"], inputs["var_a"])
    w1b, b1b = _fold_bn(inputs["w1b"], inputs["b1b"], inputs["gamma_b"],
                        inputs["beta_b"], inputs["mean_b"], inputs["var_b"])
    outw = np.zeros((DFC, 128), np.float32)
    outw[:, :OUTC] = np.asarray(inputs["out_w"], np.float32)
    outb = np.zeros((128,), np.float32)
    outb[:OUTC] = np.asarray(inputs["out_b"], np.float32)

    weights = {
        "w1a": np.ascontiguousarray(w1a),
        "w2a": np.asarray(inputs["w2a"], np.float32).astype(BF16),
        "w1b": np.ascontiguousarray(w1b),
        "w2b": np.asarray(inputs["w2b"], np.float32).astype(BF16),
        "fcw": np.asarray(inputs["fc_w"], np.float32).astype(BF16),
        "l1w": np.asarray(inputs["l1_w"], np.float32).astype(BF16),
        "l2w": np.asarray(inputs["l2_w"], np.float32).astype(BF16),
        "outw": outw.astype(BF16),
        "b1a": _bias_tile(b1a, 4),
        "b2a": _bias_tile(np.asarray(inputs["b2a"], np.float32), 4),
        "b1b": _bias_tile(b1b, 4),
        "b2b": _bias_tile(np.asarray(inputs["b2b"], np.float32), 4),
        "fcb": _bias_tile(np.asarray(inputs["fc_b"], np.float32), 16),
        "l1b": _bias_tile(np.asarray(inputs["l1_b"], np.float32), 32),
        "l2b": _bias_tile(np.asarray(inputs["l2_b"], np.float32), 16),
        "outb": _bias_tile(outb, 1),
    }

    # ---- compile (cached on the padded tile counts) ----
    if (t_lo, t_hi) not in _PROGRAM_CACHE:
        _PROGRAM_CACHE[(t_lo, t_hi)] = build_program(t_lo, t_hi)
    nc = _PROGRAM_CACHE[(t_lo, t_hi)]

    in_maps = []
    for c in range(N_CORES):
        wsl = slice(c * NWIN, (c + 1) * NWIN)
        m = dict(weights)
        m["xg"] = xg
        m["xt"] = np.ascontiguousarray(
            xt_all[:, c * NODES:(c + 1) * NODES])
        m["gilo"] = np.ascontiguousarray(gilo[wsl])
        m["gihi"] = np.ascontiguousarray(gihi[wsl])
        m["doff"] = np.ascontiguousarray(doff[wsl])
        in_maps.append(m)

    trace = bool(os.environ.get("BASS_TRACE"))
    if trace:
        _install_ntff_shim()
    res = run_bass_kernel_spmd(nc, in_maps, list(range(N_CORES)),
                               trace=trace)
    LAST_RESULT = res

    out = np.empty((N_NODES, OUTC), np.float32)
    for c in range(N_CORES):
        yc = res.results[c]["y"]          # [128, NODES] fp32
        lo = c * NODES
        hi = min((c + 1) * NODES, N_NODES)
        out[lo:hi] = yc[:OUTC, :hi - lo].T
    return out


# revision 12
# speedup vs baseline: 1.5610x; 1.5610x over previous
"""GIN message-passing + MLP head, 8-core SPMD Trainium2 (Bass/Tile) kernel.

Strategy:
- Nodes sharded by dst across 8 cores (7680 padded nodes/core). Full x
  (bf16) replicated per core for edge gathers -> no collectives needed.
- agg = segment_sum(x[src], dst) computed ONCE (shared by both GIN convs):
  edges sorted by 512-node dst window; x[src] rows fetched with gpsimd
  dma_gather (int16 idx => split lo/hi src halves); scatter-add done as
  PE matmuls against a one-hot dst-offset matrix S built on DVE.
- MLP head: BN folded into first conv weights on host; bf16 matmuls with
  fp32 PSUM accumulation; ACT-engine epilogues (bias + Relu/Lrelu/Sigmoid);
  activations stored feature-major [feat, nodes]; big intermediates staged
  through internal DRAM; l1/l2 weights SBUF-resident per output half.
"""
import os
import sys
import types
from contextlib import ExitStack

import numpy as np
import ml_dtypes

import concourse.bass as bass
import concourse.tile as tile
from concourse import bacc, mybir
from concourse.bass_utils import run_bass_kernel_spmd

BF16 = ml_dtypes.bfloat16

# ---- problem constants (hardcoded; must match the reference) ----
N_NODES = 60000
N_EDGES = 960000
C = 128               # IN_C
HID = 512
DCAT = 1024
DFC = 2048
DL1 = 4096
OUTC = 64
BN_EPS = 1e-5
LEAKY = 0.01

N_CORES = 8
NODES = 7680          # padded nodes per core
NPAD = N_CORES * NODES  # 61440
WIN = 512             # dst window (node tile) size
NWIN = NODES // WIN   # 15 windows (= node tiles) per core
NWIN_G = NPAD // WIN  # 120 global windows
LO_ROWS = 32768       # src < LO_ROWS gathers from x[0:LO_ROWS]

FP32 = mybir.dt.float32
BF = mybir.dt.bfloat16
F8 = mybir.dt.float8e4
I16 = mybir.dt.int16
E4 = ml_dtypes.float8_e4m3
WSC = 8192.0              # fp8 weight pre-scale (folded out in ACT epilogue)
AF = mybir.ActivationFunctionType

LAST_RESULT = None          # test harness introspection
_PROGRAM_CACHE = {}
K_PHASES = int(os.environ.get("K_PHASES", "6"))  # debug: stop after phase N


def _install_ntff_shim():
    """antenv.axon_hooks is missing in this image; shim it so trace=True
    (BASS_TRACE=1) can capture NTFF profiles. No-op if already present."""
    try:
        import antenv.axon_hooks  # noqa: F401
        return
    except ImportError:
        pass
    try:
        import antenv
        from trn_agent_boot.trn_boot import _ntff_profile_via_ctypes
        mod = types.ModuleType("antenv.axon_hooks")
        mod._hook = _ntff_profile_via_ctypes("/opt/axon/libaxon_pjrt.so")
        mod.get_axon_ntff_profile_hook = lambda: mod._hook
        mod.set_axon_ntff_profile_hook = lambda h: setattr(mod, "_hook", h)
        sys.modules["antenv.axon_hooks"] = mod
        antenv.axon_hooks = mod
    except Exception:
        pass


# ---------------------------------------------------------------------------
# device program
# ---------------------------------------------------------------------------

def build_program(t_lo: int, t_hi: int):
    nc = bacc.Bacc("TRN2", target_bir_lowering=False, debug=False,
                   num_devices=N_CORES, num_swdge_queues=4)
    T = t_lo + t_hi

    # ---- I/O ----
    xg = nc.dram_tensor("xg", [NPAD, C], BF, kind="ExternalInput").ap()
    xt = nc.dram_tensor("xt", [C, NODES], BF, kind="ExternalInput").ap()
    gilo = nc.dram_tensor("gilo", [NWIN, 128, t_lo * 8], I16,
                          kind="ExternalInput").ap()
    gihi = nc.dram_tensor("gihi", [NWIN, 128, t_hi * 8], I16,
                          kind="ExternalInput").ap()
    doff = nc.dram_tensor("doff", [NWIN, 128, T], FP32,
                          kind="ExternalInput").ap()
    w1a = nc.dram_tensor("w1a", [C, HID], BF, kind="ExternalInput").ap()
    w2a = nc.dram_tensor("w2a", [HID, HID], BF, kind="ExternalInput").ap()
    w1b = nc.dram_tensor("w1b", [C, HID], BF, kind="ExternalInput").ap()
    w2b = nc.dram_tensor("w2b", [HID, HID], BF, kind="ExternalInput").ap()
    fcw = nc.dram_tensor("fcw", [DCAT, DFC], BF, kind="ExternalInput").ap()
    l1w = nc.dram_tensor("l1w", [DFC, DL1], F8, kind="ExternalInput").ap()
    l2w = nc.dram_tensor("l2w", [DL1, DFC], F8, kind="ExternalInput").ap()
    outw = nc.dram_tensor("outw", [DFC, 128], BF, kind="ExternalInput").ap()
    # biases, per-partition layout [128, out_blocks]
    b1a = nc.dram_tensor("b1a", [128, 4], FP32, kind="ExternalInput").ap()
    b2a = nc.dram_tensor("b2a", [128, 4], FP32, kind="ExternalInput").ap()
    b1b = nc.dram_tensor("b1b", [128, 4], FP32, kind="ExternalInput").ap()
    b2b = nc.dram_tensor("b2b", [128, 4], FP32, kind="ExternalInput").ap()
    fcb = nc.dram_tensor("fcb", [128, 16], FP32, kind="ExternalInput").ap()
    l1b = nc.dram_tensor("l1b", [128, 32], FP32, kind="ExternalInput").ap()
    l2b = nc.dram_tensor("l2b", [128, 16], FP32, kind="ExternalInput").ap()
    outb = nc.dram_tensor("outb", [128, 1], FP32, kind="ExternalInput").ap()

    y = nc.dram_tensor("y", [128, NODES], FP32, kind="ExternalOutput").ap()

    # internal DRAM staging, feature-block-major [kb, 128, nodes]
    cat = nc.dram_tensor("cat", [8, 128, NODES], BF).ap()
    g1 = nc.dram_tensor("g1", [16, 128, NODES], F8).ap()
    g2 = nc.dram_tensor("g2", [32, 128, NODES], F8).ap()
    g3 = nc.dram_tensor("g3", [16, 128, NODES], BF).ap()

    def wview(ap, kb):
        # [K, O] dram -> [128, kb, O] AP (partition = K within block)
        return ap.rearrange("(kb p) o -> p kb o", p=128, kb=kb)

    with tile.TileContext(nc) as tc, ExitStack() as big:
        ps_pool = big.enter_context(tc.tile_pool(name="ps", bufs=8,
                                                 space="PSUM"))
        bias_pool = big.enter_context(tc.tile_pool(name="bias", bufs=1))
        otile_pool = big.enter_context(tc.tile_pool(name="otile", bufs=6))

        def load_bias(ap, nb):
            t = bias_pool.tile([128, nb], FP32, tag=ap.name)
            nc.sync.dma_start(t[:], ap[:])
            return t

        b1a_t = load_bias(b1a, 4)
        b2a_t = load_bias(b2a, 4)
        b1b_t = load_bias(b1b, 4)
        b2b_t = load_bias(b2b, 4)
        fcb_t = load_bias(fcb, 16)
        l1b_t = load_bias(l1b, 32)
        l2b_t = load_bias(l2b, 16)
        outb_t = load_bias(outb, 1)

        with ExitStack() as ph12:
            persist = ph12.enter_context(tc.tile_pool(name="persist",
                                                      bufs=1))
            xt_t = persist.tile([128, NODES], BF, tag="xt")
            nc.sync.dma_start(xt_t[:], xt[:])
            h_t = persist.tile([128, NODES], BF, tag="h")

            # ======== phase 1: aggregation  h = x + scatter_add ========
            for _ph1 in ([0] if K_PHASES >= 1 else []):
             with ExitStack() as ph:
                ipool = ph.enter_context(tc.tile_pool(name="aggidx",
                                                      bufs=3))
                gpool = ph.enter_context(tc.tile_pool(name="gath", bufs=3))
                spool = ph.enter_context(tc.tile_pool(name="onehot",
                                                      bufs=6))
                cpool = ph.enter_context(tc.tile_pool(name="aggconst",
                                                      bufs=1))

                qn = [0]
                iota_t = cpool.tile([128, WIN], I16)
                nc.gpsimd.iota(iota_t[:], pattern=[[1, WIN]], base=0,
                               channel_multiplier=0)

                for w in range(NWIN):
                    il = ipool.tile([128, t_lo * 8], I16, tag="il")
                    nc.sync.dma_start(il[:], gilo[w])
                    ih = ipool.tile([128, t_hi * 8], I16, tag="ih")
                    nc.sync.dma_start(ih[:], gihi[w])
                    do_t = ipool.tile([128, T], FP32, tag="doff")
                    nc.sync.dma_start(do_t[:], doff[w])

                    # SWDGE ring holds 1024 descriptors -> chunk gathers
                    # to <=8 edge-tiles (1024 idxs) per instruction.
                    gl = gpool.tile([128, t_lo, C], BF, tag="gl")
                    for c0 in range(0, t_lo, 8):
                        c1 = min(c0 + 8, t_lo)
                        ni = (c1 - c0) * 128
                        nc.gpsimd.dma_gather(
                            gl[:, c0:c1, :], xg[0:LO_ROWS, :],
                            il[:, c0 * 8:c1 * 8],
                            num_idxs=ni, num_idxs_reg=ni, elem_size=C,
                            queue_num=qn[0] % 4)
                        qn[0] += 1
                    gh = gpool.tile([128, t_hi, C], BF, tag="gh")
                    for c0 in range(0, t_hi, 8):
                        c1 = min(c0 + 8, t_hi)
                        ni = (c1 - c0) * 128
                        nc.gpsimd.dma_gather(
                            gh[:, c0:c1, :], xg[LO_ROWS:NPAD, :],
                            ih[:, c0 * 8:c1 * 8],
                            num_idxs=ni, num_idxs_reg=ni, elem_size=C,
                            queue_num=qn[0] % 4)
                        qn[0] += 1

                    ps = ps_pool.tile([128, WIN], FP32, tag="ps")
                    for t in range(T):
                        s = spool.tile([128, WIN], BF, tag="s")
                        nc.vector.tensor_scalar(
                            s[:], iota_t[:], do_t[:, t:t + 1], None,
                            mybir.AluOpType.is_equal)
                        g = gl[:, t, :] if t < t_lo else gh[:, t - t_lo, :]
                        nc.tensor.matmul(ps[:], g, s[:],
                                         start=(t == 0), stop=(t == T - 1))
                    nc.vector.tensor_add(h_t[:, w * WIN:(w + 1) * WIN],
                                         xt_t[:, w * WIN:(w + 1) * WIN],
                                         ps[:])

            # ======== phase 2: the two GIN conv MLPs -> cat ========
            for _ph2 in ([0] if K_PHASES >= 2 else []):
             with ExitStack() as ph:
                wpool = ph.enter_context(tc.tile_pool(name="convw", bufs=1))
                apool = ph.enter_context(tc.tile_pool(name="convact",
                                                      bufs=2))

                w1a_t = wpool.tile([128, HID], BF, tag="w1a")
                nc.sync.dma_start(w1a_t[:], w1a[:])
                w2a_t = wpool.tile([128, 4, HID], BF, tag="w2a")
                nc.sync.dma_start(w2a_t[:], wview(w2a, 4))
                w1b_t = wpool.tile([128, HID], BF, tag="w1b")
                nc.sync.dma_start(w1b_t[:], w1b[:])
                w2b_t = wpool.tile([128, 4, HID], BF, tag="w2b")
                nc.sync.dma_start(w2b_t[:], wview(w2b, 4))

                for nt in range(NWIN):
                    hs = h_t[:, nt * WIN:(nt + 1) * WIN]
                    for conv, (w1_t, w2_t, bb1, bb2) in enumerate(
                            [(w1a_t, w2a_t, b1a_t, b2a_t),
                             (w1b_t, w2b_t, b1b_t, b2b_t)]):
                        xa = apool.tile([128, 4, WIN], BF, tag="x1a")
                        for of in range(4):
                            ps = ps_pool.tile([128, WIN], FP32, tag="ps")
                            nc.tensor.matmul(
                                ps[:],
                                w1_t[:, of * 128:(of + 1) * 128],
                                hs, start=True, stop=True)
                            nc.scalar.activation(xa[:, of, :], ps[:],
                                                 AF.Relu,
                                                 bias=bb1[:, of:of + 1])
                        for of in range(4):
                            ps = ps_pool.tile([128, WIN], FP32, tag="ps")
                            for kb in range(4):
                                nc.tensor.matmul(
                                    ps[:],
                                    w2_t[:, kb, of * 128:(of + 1) * 128],
                                    xa[:, kb, :],
                                    start=(kb == 0), stop=(kb == 3))
                            ot = otile_pool.tile([128, WIN], BF, tag="ot")
                            nc.scalar.activation(ot[:], ps[:], AF.Relu,
                                                 bias=bb2[:, of:of + 1])
                            nc.sync.dma_start(
                                cat[conv * 4 + of, :,
                                    nt * WIN:(nt + 1) * WIN], ot[:])

        # big weight pool created only now so agg/conv phases keep SBUF room
        bigw = big.enter_context(tc.tile_pool(name="bigw", bufs=2))
        linpool = None

        # ============ generic streamed linear layer ============
        def _epilogue(ps, dst, of_g, bias_t, func, alpha, nts, out_fp32_to,
                      scale=1.0, out_dt=BF):
            if out_fp32_to is not None:
                ot = otile_pool.tile([128, WIN], FP32, tag="yt")
                nc.scalar.activation(ot[:], ps[:], func,
                                     bias=bias_t[:, of_g:of_g + 1],
                                     alpha=alpha, scale=scale)
                nc.sync.dma_start(out_fp32_to[:, nts], ot[:])
            else:
                tag = "ot8" if out_dt == F8 else "ot"
                ot = otile_pool.tile([128, WIN], out_dt, tag=tag)
                nc.scalar.activation(ot[:], ps[:], func,
                                     bias=bias_t[:, of_g:of_g + 1],
                                     alpha=alpha, scale=scale)
                nc.sync.dma_start(dst[of_g, :, nts], ot[:])

        def linear(src, dst, w_t, bias_t, of_base, kb_n, of_n, func,
                   in_tag, kgroups=1, alpha=0.0, out_fp32_to=None,
                   out_dt=BF):
            kb_per_g = kb_n // kgroups
            for nt in range(NWIN):
                nts = slice(nt * WIN, (nt + 1) * WIN)
                if kgroups == 1:
                    it = linpool.tile([128, kb_n, WIN], BF, tag=in_tag)
                    nc.sync.dma_start(it[:],
                                      src[:, :, nts].transpose([1, 0, 2]))
                    for of in range(of_n):
                        ps = ps_pool.tile([128, WIN], FP32, tag="ps")
                        for kb in range(kb_n):
                            nc.tensor.matmul(
                                ps[:],
                                w_t[:, kb, of * 128:(of + 1) * 128],
                                it[:, kb, :],
                                start=(kb == 0), stop=(kb == kb_n - 1))
                        _epilogue(ps, dst, of_base + of, bias_t, func,
                                  alpha, nts, out_fp32_to, out_dt=out_dt)
                else:
                    pss = [ps_pool.tile([128, WIN], FP32, tag="ps",
                                        name=f"pss{of}")
                           for of in range(of_n)]
                    for kg in range(kgroups):
                        it = linpool.tile([128, kb_per_g, WIN], BF,
                                          tag=in_tag)
                        ksl = slice(kg * kb_per_g, (kg + 1) * kb_per_g)
                        nc.sync.dma_start(
                            it[:], src[ksl, :, nts].transpose([1, 0, 2]))
                        for of in range(of_n):
                            for kb in range(kb_per_g):
                                kbg = kg * kb_per_g + kb
                                nc.tensor.matmul(
                                    pss[of][:],
                                    w_t[:, kbg, of * 128:(of + 1) * 128],
                                    it[:, kb, :],
                                    start=(kbg == 0),
                                    stop=(kbg == kb_n - 1))
                    for of in range(of_n):
                        _epilogue(pss[of], dst, of_base + of, bias_t, func,
                                  alpha, nts, out_fp32_to, out_dt=out_dt)

        def linear8(src, dst, w_t, bias_t, of_base, jp_n, of_n, func,
                    in_tag, out_dt=BF):
            """fp8 DoubleRow layer: w_t [128, jp_n, 2, of_n*128] fp8,
            src dram [jp_n*2, 128, NODES] fp8. Epilogue scales by 1/WSC."""
            src_v = src.rearrange("(j two) p n -> p j two n", two=2)
            for nt in range(NWIN):
                nts = slice(nt * WIN, (nt + 1) * WIN)
                it = linpool.tile([128, jp_n, 2, WIN], F8, tag=in_tag)
                nc.sync.dma_start(it[:], src_v[:, :, :, nts])
                for of in range(of_n):
                    ps = ps_pool.tile([128, WIN], FP32, tag="ps")
                    for j in range(jp_n):
                        nc.tensor.matmul(
                            ps[:],
                            w_t[:, j, :, of * 128:(of + 1) * 128],
                            it[:, j, :, :],
                            start=(j == 0), stop=(j == jp_n - 1),
                            perf_mode=mybir.MatmulPerfMode.DoubleRow)
                    _epilogue(ps, dst, of_base + of, bias_t, func, 0.0,
                              nts, None, scale=1.0 / WSC, out_dt=out_dt)

        # ---- fc: cat[1024] -> g1[2048], leaky relu ----
        for _ph3 in ([0] if K_PHASES >= 3 else []):
         with ExitStack() as ph:
            linpool = ph.enter_context(tc.tile_pool(name="fcin", bufs=2))
            fcw_t = bigw.tile([128, 8, DFC], BF, tag="bigw")
            nc.sync.dma_start(fcw_t[:], wview(fcw, 8))
            linear(cat, g1, fcw_t, fcb_t, 0, 8, 16, AF.Lrelu, "fcin",
                   alpha=LEAKY, out_dt=F8)

        # ---- l1: g1[2048] -> g2[4096], linear (two output halves) ----
        for _ph4 in ([0] if K_PHASES >= 4 else []):
         with ExitStack() as ph:
            linpool = ph.enter_context(tc.tile_pool(name="l1in", bufs=3))
            l1wv = l1w.rearrange("(j two p) o -> p j two o", p=128, two=2)
            for hx in range(2):
                wt = bigw.tile([128, 8, 2, 2048], F8, tag="bigw")
                nc.sync.dma_start(
                    wt[:], l1wv[:, :, :, hx * 2048:(hx + 1) * 2048])
                linear8(g1, g2, wt, l1b_t, hx * 16, 8, 16, AF.Identity,
                        "l1in", out_dt=F8)

        # ---- l2: g2[4096] -> g3[2048], linear (two output halves) ----
        for _ph5 in ([0] if K_PHASES >= 5 else []):
         with ExitStack() as ph:
            linpool = ph.enter_context(tc.tile_pool(name="l2in", bufs=3))
            l2wv = l2w.rearrange("(j two p) o -> p j two o", p=128, two=2)
            for hx in range(2):
                wt = bigw.tile([128, 16, 2, 1024], F8, tag="bigw")
                nc.sync.dma_start(
                    wt[:], l2wv[:, :, :, hx * 1024:(hx + 1) * 1024])
                linear8(g2, g3, wt, l2b_t, hx * 8, 16, 8, AF.Identity,
                        "l2in", out_dt=BF)

        # ---- out: g3[2048] -> y[64(pad128)], sigmoid, fp32 ----
        for _ph6 in ([0] if K_PHASES >= 6 else []):
         with ExitStack() as ph:
            linpool = ph.enter_context(tc.tile_pool(name="outin", bufs=2))
            ow_t = bigw.tile([128, 16, 128], BF, tag="outw")
            nc.sync.dma_start(ow_t[:], wview(outw, 16))
            linear(g3, None, ow_t, outb_t, 0, 16, 1, AF.Sigmoid, "outin",
                   out_fp32_to=y)

    nc.compile()
    return nc


# ---------------------------------------------------------------------------
# host-side sharding / preprocessing
# ---------------------------------------------------------------------------

def _fold_bn(w1, b1, gamma, beta, mean, var):
    s = (np.asarray(gamma, np.float64)
         / np.sqrt(np.asarray(var, np.float64) + BN_EPS))
    w = np.asarray(w1, np.float64) * s[None, :]
    b = (np.asarray(b1, np.float64) - np.asarray(mean, np.float64)) * s \
        + np.asarray(beta, np.float64)
    return w.astype(BF16), b.astype(np.float32)


def _bias_tile(b, nb):
    # [out] -> [128, nb] with b[of*128+p] at [p, of]
    bb = np.asarray(b, np.float32)
    return np.ascontiguousarray(bb.reshape(nb, 128).T)


def _idx_layout(arr, nwin):
    # [nwin, T*128] int16 -> [nwin, 128, T*8]: idx i at partition i%16,
    # col i//16, replicated over the 8 gpsimd-core partition groups.
    t8 = arr.shape[1] // 16
    a = arr.reshape(nwin, t8, 16).transpose(0, 2, 1)       # [nwin, 16, T*8]
    return np.ascontiguousarray(np.tile(a, (1, 8, 1)))     # [nwin, 128, T*8]


def kernel(**inputs) -> np.ndarray:
    global LAST_RESULT
    x = np.asarray(inputs["x"], np.float32)
    edge_index = np.asarray(inputs["edge_index"])

    # ---- graph partition: sort edges by (global dst window, src half) ----
    src = edge_index[0].astype(np.int64)
    dst = edge_index[1].astype(np.int64)
    win = dst // WIN                      # 0..119 global window
    half = (src >= LO_ROWS).astype(np.int64)
    key = win * 2 + half
    order = np.argsort(key, kind="stable")
    src_s = src[order]
    dst_s = dst[order]
    key_s = key[order]
    counts = np.bincount(key_s, minlength=NWIN_G * 2)
    starts = np.zeros(NWIN_G * 2 + 1, np.int64)
    np.cumsum(counts, out=starts[1:])
    rank = np.arange(N_EDGES, dtype=np.int64) - starts[key_s]

    cnt_lo = counts[0::2]
    cnt_hi = counts[1::2]
    t_lo = int(np.ceil(cnt_lo.max() / 128))
    t_hi = int(np.ceil(cnt_hi.max() / 128))
    T = t_lo + t_hi

    gidx_lo = np.zeros((NWIN_G, t_lo * 128), np.int16)
    gidx_hi = np.zeros((NWIN_G, t_hi * 128), np.int16)
    dstoff = np.full((NWIN_G, T * 128), -1.0, np.float32)

    is_lo = key_s % 2 == 0
    wl = key_s[is_lo] // 2
    gidx_lo[wl, rank[is_lo]] = src_s[is_lo].astype(np.int16)
    dstoff[wl, rank[is_lo]] = (dst_s[is_lo] % WIN).astype(np.float32)
    wh = key_s[~is_lo] // 2
    gidx_hi[wh, rank[~is_lo]] = (src_s[~is_lo] - LO_ROWS).astype(np.int16)
    dstoff[wh, t_lo * 128 + rank[~is_lo]] = \
        (dst_s[~is_lo] % WIN).astype(np.float32)

    gilo = _idx_layout(gidx_lo, NWIN_G)
    gihi = _idx_layout(gidx_hi, NWIN_G)
    # dstoff: edge t*128+j -> [w, j, t]
    doff = np.ascontiguousarray(
        dstoff.reshape(NWIN_G, T, 128).transpose(0, 2, 1))

    # ---- node features ----
    xpad = np.zeros((NPAD, C), np.float32)
    xpad[:N_NODES] = x
    xg = xpad.astype(BF16)
    xt_all = np.ascontiguousarray(xg.T)   # [128, NPAD] bf16

    # ---- weights (fold BN, cast bf16) ----
    w1a, b1a = _fold_bn(inputs["w1a"], inputs["b1a"], inputs["gamma_a"],
                        inputs["beta_a"], inputs["mean_a"], inputs["var_a"])
    w1b, b1b = _fold_bn(inputs["w1b"], inputs["b1b"], inputs["gamma_b"],
                        inputs["beta_b"], inputs["mean_b"], inputs["var_b"])
    outw = np.zeros((DFC, 128), np.float32)
    outw[:, :OUTC] = np.asarray(inputs["out_w"], np.float32)
    outb = np.zeros((128,), np.float32)
    outb[:OUTC] = np.asarray(inputs["out_b"], np.float32)

    weights = {
        "w1a": np.ascontiguousarray(w1a),
        "w2a": np.asarray(inputs["w2a"], np.float32).astype(BF16),
        "w1b": np.ascontiguousarray(w1b),
        "w2b": np.asarray(inputs["w2b"], np.float32).astype(BF16),
        "fcw": np.asarray(inputs["fc_w"], np.float32).astype(BF16),
        "l1w": (np.asarray(inputs["l1_w"], np.float32) * WSC).astype(E4),
        "l2w": (np.asarray(inputs["l2_w"], np.float32) * WSC).astype(E4),
        "outw": outw.astype(BF16),
        "b1a": _bias_tile(b1a, 4),
        "b2a": _bias_tile(np.asarray(inputs["b2a"], np.float32), 4),
        "b1b": _bias_tile(b1b, 4),
        "b2b": _bias_tile(np.asarray(inputs["b2b"], np.float32), 4),
        "fcb": _bias_tile(np.asarray(inputs["fc_b"], np.float32), 16),
        "l1b": _bias_tile(np.asarray(inputs["l1_b"], np.float32), 32),
        "l2b": _bias_tile(np.asarray(inputs["l2_b"], np.float32), 16),
        "outb": _bias_tile(outb, 1),
    }

    # ---- compile (cached on the padded tile counts) ----
    if (t_lo, t_hi) not in _PROGRAM_CACHE:
        _PROGRAM_CACHE[(t_lo, t_hi)] = build_program(t_lo, t_hi)
    nc = _PROGRAM_CACHE[(t_lo, t_hi)]

    in_maps = []
    for c in range(N_CORES):
        wsl = slice(c * NWIN, (c + 1) * NWIN)
        m = dict(weights)
        m["xg"] = xg
        m["xt"] = np.ascontiguousarray(
            xt_all[:, c * NODES:(c + 1) * NODES])
        m["gilo"] = np.ascontiguousarray(gilo[wsl])
        m["gihi"] = np.ascontiguousarray(gihi[wsl])
        m["doff"] = np.ascontiguousarray(doff[wsl])
        in_maps.append(m)

    trace = bool(os.environ.get("BASS_TRACE"))
    if trace:
        _install_ntff_shim()
    res = run_bass_kernel_spmd(nc, in_maps, list(range(N_CORES)),
                               trace=trace)
    LAST_RESULT = res

    out = np.empty((N_NODES, OUTC), np.float32)
    for c in range(N_CORES):
        yc = res.results[c]["y"]          # [128, NODES] fp32
        lo = c * NODES
        hi = min((c + 1) * NODES, N_NODES)
        out[lo:hi] = yc[:OUTC, :hi - lo].T
    return out
